# revision 1
# baseline (speedup 1.0000x reference)
"""Self-contained TRN2 Bass kernel for 16-head MHA (B=2, T=2048, D=1024),
head-parallel across 8 NeuronCores (2 heads per core).

kernel(**inputs) takes the FULL fp32 inputs of reference.setup_inputs() and
returns the FULL [2, 2048, 1024] fp32 output.  Host-side prep: q/k/v are
transposed to [1024, 4096] bf16 (shared by all cores); each core gets its
128-column slice of Wq/Wk/Wv (and 128-row slice of Wo) in bf16.  Each core
computes its two heads end-to-end (QKV projections, softmax attention with
row-group-packed score matmuls, ones-augmented PV for free softmax sums,
output projection) and DMAs a rank-128 partial of the output back; the host
sums the 8 partials and adds the output bias.
"""

import numpy as np

import concourse.bass as bass
import concourse.mybir as mybir
import concourse.tile as tile
from concourse import bacc

FP32 = mybir.dt.float32
BF16 = mybir.dt.bfloat16

D = 1024          # model dim
N = 4096          # B*T tokens
B = 2
T = 2048
PH = 128          # per-core projection dims (2 heads x 64)
DH = 64           # head dim
KC = 8            # contraction chunks (1024 / 128)
NTC = N // 128    # 32 token chunks of 128
SCALE = 0.125     # 1/sqrt(64)

ACT_EXP = mybir.ActivationFunctionType.Exp


def build(nc=None):
    if nc is None:
        nc = bacc.Bacc(
            "TRN2",
            target_bir_lowering=False,
            debug=False,
            enable_asserts=False,
            num_devices=8,
        )

    qT = nc.dram_tensor("qT", [D, N], BF16, kind="ExternalInput")
    kT = nc.dram_tensor("kT", [D, N], BF16, kind="ExternalInput")
    vT = nc.dram_tensor("vT", [D, N], BF16, kind="ExternalInput")
    wq = nc.dram_tensor("wq", [D, PH], BF16, kind="ExternalInput")
    wk = nc.dram_tensor("wk", [D, PH], BF16, kind="ExternalInput")
    wv = nc.dram_tensor("wv", [D, PH], BF16, kind="ExternalInput")
    wo = nc.dram_tensor("wo", [PH, D], BF16, kind="ExternalInput")
    bq = nc.dram_tensor("bq", [PH, 1], FP32, kind="ExternalInput")
    bk = nc.dram_tensor("bk", [PH, 1], FP32, kind="ExternalInput")
    bv = nc.dram_tensor("bv", [PH, 1], FP32, kind="ExternalInput")
    out = nc.dram_tensor("out", [N, D], FP32, kind="ExternalOutput")

    with tile.TileContext(nc) as tc:
        _emit(nc, tc, qT, kT, vT, wq, wk, wv, wo, bq, bk, bv, out)

    nc.compile()
    return nc


class _Ctx:
    pass


def _emit(nc, tc, qT, kT, vT, wq, wk, wv, wo, bq, bk, bv, out):
    from contextlib import ExitStack

    E = _Ctx()
    E.nc = nc
    E.pending = []

    ctxmgr = ExitStack()
    with ctxmgr:
        const_pool = ctxmgr.enter_context(tc.tile_pool(name="const", bufs=1))
        E.xt_pool = ctxmgr.enter_context(tc.tile_pool(name="xt", bufs=18))
        big_pool = ctxmgr.enter_context(tc.tile_pool(name="big", bufs=1))
        E.pt_pool = ctxmgr.enter_context(tc.tile_pool(name="pt", bufs=17))
        E.bc_pool = ctxmgr.enter_context(tc.tile_pool(name="bc", bufs=4))
        E.ostg_pool = ctxmgr.enter_context(tc.tile_pool(name="ostg", bufs=4))
        # PSUM: shared proj/outproj pool 2 banks + st 4 + ctx 2 = 8
        E.po_ps = ctxmgr.enter_context(
            tc.tile_pool(name="po_ps", bufs=2, space="PSUM"))
        E.st_ps = ctxmgr.enter_context(
            tc.tile_pool(name="st_ps", bufs=2, space="PSUM"))
        E.ctx_ps = ctxmgr.enter_context(
            tc.tile_pool(name="ctx_ps", bufs=2, space="PSUM"))

        # --- weights / consts to SBUF ---
        wq_sb = const_pool.tile([128, KC, PH], BF16, tag="wq")
        wk_sb = const_pool.tile([128, KC, PH], BF16, tag="wk")
        wv_sb = const_pool.tile([128, KC, PH], BF16, tag="wv")
        E.wo_sb = const_pool.tile([128, D], BF16, tag="wo")
        bq_sb = const_pool.tile([128, 1], FP32, tag="bq")
        bk_sb = const_pool.tile([128, 1], FP32, tag="bk")
        E.bv_sb = const_pool.tile([128, 1], FP32, tag="bv")
        nc.sync.dma_start(wk_sb[:], wk.ap().rearrange("(c p) m -> p c m", p=128))
        nc.sync.dma_start(wq_sb[:], wq.ap().rearrange("(c p) m -> p c m", p=128))
        nc.sync.dma_start(wv_sb[:], wv.ap().rearrange("(c p) m -> p c m", p=128))
        nc.sync.dma_start(E.wo_sb[:], wo.ap())
        nc.sync.dma_start(bq_sb[:], bq.ap())
        nc.sync.dma_start(bk_sb[:], bk.ap())
        nc.sync.dma_start(E.bv_sb[:], bv.ap())

        # persistent activations
        E.qT_sb = big_pool.tile([128, N], BF16, tag="qTsb")
        E.kT_sb = big_pool.tile([128, N], BF16, tag="kTsb")
        # v_aug pair layout: [tok part, 32 tok chunks, 130]; per head h the
        # PV stationary operand is vp[:, chunk, 65h : 65h+65] = [v_h | ones]
        E.vp = big_pool.tile([128, NTC, 130], BF16, tag="vp")
        E.vT_sb = big_pool.tile([128, N], BF16, tag="vTsb")
        E.ctxT = big_pool.tile([128, N], BF16, tag="ctxT")

        nc.gpsimd.memset(E.vp[:, :, 64], 1.0)
        nc.gpsimd.memset(E.vp[:, :, 129], 1.0)

        E.identity = const_pool.tile([128, 128], BF16, tag="ident")
        from concourse.masks import make_identity
        make_identity(nc, E.identity[:])

        def dma_in(nm, xdram, b):
            lst = []
            for kc in range(KC):
                xt = E.xt_pool.tile(
                    [128, T], BF16, tag="xt", name=f"xt_{nm}{b}{kc}")
                nc.sync.dma_start(
                    xt[:], xdram.ap()[kc * 128:(kc + 1) * 128, b * T:(b + 1) * T])
                lst.append(xt)
            return lst

        def proj4(xts, wsb, bias_sb, dstT, b, ts=range(4), drain_act=False):
            for t in ts:
                _proj_chunk(E, xts, wsb, bias_sb, dstT, b * T, t, drain_act)

        def vproj(xts, b):
            proj4(xts, wv_sb, None, E.vT_sb, b)
            for tloc in range(16):
                tcid = b * 16 + tloc
                tr = E.po_ps.tile(
                    [128, 128], BF16, tag="po", name=f"tr{tcid}")
                nc.tensor.transpose(
                    tr[:], E.vT_sb[:, tcid * 128:(tcid + 1) * 128], E.identity[:])
                nc.vector.tensor_copy(E.vp[:, tcid, 0:64], tr[:, 0:64])
                nc.vector.tensor_copy(E.vp[:, tcid, 65:129], tr[:, 64:128])

        def group(b, tqc, mid_cb=None, defer=2):
            pend = _attention_group(E, b, tqc, mid_cb)
            while len(E.pending) >= defer:
                _norm_outproj(E, *E.pending.pop(0), out)
            E.pending.append(pend)

        # batch-0 inputs + K/Q projections up front; V is emitted after the
        # first attention group so the PE stream does not stall on vT DMA.
        # batch-1 inputs/projections are staggered into batch-0's ACT-bound
        # attention groups so their DMA + PE work hide in the slack.
        xk0 = dma_in("k", kT, 0)
        xq0 = dma_in("q", qT, 0)
        xv0 = dma_in("v", vT, 0)
        proj4(xk0, wk_sb, bk_sb, E.kT_sb, 0, drain_act=True)
        proj4(xq0, wq_sb, bq_sb, E.qT_sb, 0, drain_act=True)
        xk1 = dma_in("k", kT, 1)
        group(0, 0, mid_cb=lambda: vproj(xv0, 0))
        xq1 = dma_in("q", qT, 1)
        group(0, 1)
        proj4(xk1, wk_sb, bk_sb, E.kT_sb, 1, ts=(0, 1))
        xv1 = dma_in("v", vT, 1)
        group(0, 2)
        proj4(xk1, wk_sb, bk_sb, E.kT_sb, 1, ts=(2, 3))
        proj4(xq1, wq_sb, bq_sb, E.qT_sb, 1, ts=(0, 1))
        group(0, 3)
        proj4(xq1, wq_sb, bq_sb, E.qT_sb, 1, ts=(2, 3))
        group(1, 0, mid_cb=lambda: vproj(xv1, 1))
        group(1, 1)
        group(1, 2, defer=1)
        group(1, 3, defer=1)
        while E.pending:
            _norm_outproj(E, *E.pending.pop(0), out)


def _proj_chunk(E, xts, wsb, bias_sb, dstT, btok, t, drain_act=False):
    """One 512-token projection chunk: accumulate 8 kc matmuls, drain."""
    nc = E.nc
    ps = E.po_ps.tile([128, 512], FP32, tag="po", name="ps")
    for kc in range(KC):
        nc.tensor.matmul(
            ps[:],
            wsb[:, kc, :],
            xts[kc][:, t * 512:(t + 1) * 512],
            start=(kc == 0),
            stop=(kc == KC - 1),
        )
    dst = dstT[:, btok + t * 512: btok + (t + 1) * 512]
    if drain_act:
        # ScalarE drain (idle during the head phase); Identity has a free
        # per-partition bias add
        if bias_sb is not None:
            nc.scalar.activation(
                dst, ps[:], mybir.ActivationFunctionType.Identity, bias=bias_sb[:])
        else:
            nc.scalar.activation(dst, ps[:], mybir.ActivationFunctionType.Identity)
    elif bias_sb is not None:
        nc.vector.tensor_scalar_add(dst, ps[:], bias_sb[:])
    else:
        nc.vector.tensor_copy(dst, ps[:])


def _attention_group(E, b, tqc, mid_cb=None):
    """S^T/exp/PV + sums & ctx drains for one 512-token group (both heads).

    The two heads' S^T matmuls are row-group packed: head h's K=64
    contraction occupies array rows 64h..64h+63, so the pair runs
    concurrently on the PE (measured ~2.7x over sequential issue).

    With mid_cb set, all 16 ST/exp pairs are emitted first, then mid_cb()
    (used for the V projection: ScalarE stays busy on the exps while the
    PE waits for vT's DMA), then the PV accumulation.
    """
    nc = E.nc
    btok = b * T
    tq0 = btok + tqc * 512

    sums_h = [
        E.bc_pool.tile([1, 512], FP32, tag=f"sums{h}", name=f"sums{h}")
        for h in range(2)
    ]
    ctx2 = [
        E.ctx_ps.tile([65, 512], FP32, tag="ctx", name=f"ctx{h}")
        for h in range(2)
    ]

    def st_exp(tk):
        st = E.st_ps.tile([128, 2, 512], FP32, tag="st", name="st")
        for h in range(2):
            nc.tensor.matmul(
                st[:, h, :],
                E.kT_sb[h * 64:(h + 1) * 64,
                        btok + tk * 128: btok + (tk + 1) * 128],
                E.qT_sb[h * 64:(h + 1) * 64, tq0:tq0 + 512],
                start=True,
                stop=True,
            )
        pt = E.pt_pool.tile([128, 2, 512], BF16, tag="pt", name="pt")
        nc.scalar.activation(pt[:], st[:], ACT_EXP, scale=SCALE)
        return pt

    def pv(tk, pt):
        for h in range(2):
            nc.tensor.matmul(
                ctx2[h][:],
                E.vp[:, b * 16 + tk, h * 65:(h + 1) * 65],
                pt[:, h, :],
                start=(tk == 0),
                stop=(tk == 15),
            )

    if mid_cb is None:
        for tk in range(16):
            pv(tk, st_exp(tk))
    else:
        pts = [st_exp(tk) for tk in range(16)]
        mid_cb()
        for tk in range(16):
            pv(tk, pts[tk])

    for h in range(2):
        # softmax sums (PSUM row 64) -> sums tile partition 0
        nc.vector.tensor_copy(sums_h[h][0:1, :], ctx2[h][64:65, :])
        # ctx drain with bf16 cast (h1 shifts base 0 -> 64)
        nc.vector.tensor_copy(
            E.ctxT[h * 64:(h + 1) * 64, tq0:tq0 + 512], ctx2[h][0:64, :])
    return (tq0, sums_h)


def _norm_outproj(E, tq0, sums_h, out):
    """Normalization + V-bias + output projection for one 512-token group."""
    nc = E.nc
    bcast = E.bc_pool.tile([128, 512], FP32, tag="bcast")
    bcb = E.bc_pool.tile([128, 512], FP32, tag="bcb")
    nc.gpsimd.partition_broadcast(bcast[0:64, :], sums_h[0][0:1, :])
    nc.gpsimd.partition_broadcast(bcb[0:64, :], sums_h[1][0:1, :])
    nc.vector.tensor_copy(bcast[64:128, :], bcb[0:64, :])
    recipb = E.bc_pool.tile([128, 512], FP32, tag="recipb")
    nc.vector.reciprocal_approx_fast(recipb[:], bcast[:])
    nc.vector.tensor_mul(E.ctxT[:, tq0:tq0 + 512], E.ctxT[:, tq0:tq0 + 512], recipb[:])
    nc.vector.tensor_scalar_add(
        E.ctxT[:, tq0:tq0 + 512], E.ctxT[:, tq0:tq0 + 512], E.bv_sb[:])

    # output projection for these 512 tokens
    for tc4 in range(4):
        t0 = tq0 + tc4 * 128
        for half in range(2):
            ops = E.po_ps.tile([128, 512], FP32, tag="po", name="ops")
            nc.tensor.matmul(
                ops[:],
                E.ctxT[:, t0:t0 + 128],
                E.wo_sb[:, half * 512:(half + 1) * 512],
                start=True,
                stop=True,
            )
            ostg = E.ostg_pool.tile([128, 512], FP32, tag="ostg")
            nc.vector.tensor_copy(ostg[:], ops[:])
            nc.sync.dma_start(
                out.ap()[t0:t0 + 128, half * 512:(half + 1) * 512], ostg[:])


# ---------------- host-side helpers ----------------

def core_inputs(q, k, v, Wq, bq_, Wk, bk_, Wv, bv_, Wo, core):
    """Build the per-core input map (numpy, host-side shard/layout prep)."""
    import ml_dtypes

    bf16 = ml_dtypes.bfloat16
    dsl = slice(core * PH, (core + 1) * PH)
    return {
        "wq": np.ascontiguousarray(Wq[:, dsl]).astype(bf16),
        "wk": np.ascontiguousarray(Wk[:, dsl]).astype(bf16),
        "wv": np.ascontiguousarray(Wv[:, dsl]).astype(bf16),
        "wo": np.ascontiguousarray(Wo[dsl, :]).astype(bf16),
        "bq": np.ascontiguousarray(bq_[dsl]).reshape(PH, 1).astype(np.float32),
        "bk": np.ascontiguousarray(bk_[dsl]).reshape(PH, 1).astype(np.float32),
        "bv": np.ascontiguousarray(bv_[dsl]).reshape(PH, 1).astype(np.float32),
    }


def shared_inputs(q, k, v):
    import ml_dtypes

    bf16 = ml_dtypes.bfloat16
    qT_np = np.ascontiguousarray(q.reshape(N, D).T).astype(bf16)
    kT_np = np.ascontiguousarray(k.reshape(N, D).T).astype(bf16)
    vT_np = np.ascontiguousarray(v.reshape(N, D).T).astype(bf16)
    return {"qT": qT_np, "kT": kT_np, "vT": vT_np}


# ---------------- public entry point ----------------

_NC_CACHE = []


def _get_nc():
    if not _NC_CACHE:
        _NC_CACHE.append(build())
    return _NC_CACHE[0]


def kernel(q, k, v, Wq, bq, Wk, bk, Wv, bv, Wo, bo):
    from concourse import bass_utils

    q = np.asarray(q, np.float32)
    k = np.asarray(k, np.float32)
    v = np.asarray(v, np.float32)
    Wq, bq = np.asarray(Wq, np.float32), np.asarray(bq, np.float32)
    Wk, bk = np.asarray(Wk, np.float32), np.asarray(bk, np.float32)
    Wv, bv = np.asarray(Wv, np.float32), np.asarray(bv, np.float32)
    Wo, bo = np.asarray(Wo, np.float32), np.asarray(bo, np.float32)

    nc = _get_nc()
    shared = shared_inputs(q, k, v)
    in_maps = []
    for core in range(8):
        m = dict(shared)
        m.update(core_inputs(q, k, v, Wq, bq, Wk, bk, Wv, bv, Wo, core))
        in_maps.append(m)

    res = bass_utils.run_bass_kernel_spmd(nc, in_maps, core_ids=list(range(8)))

    acc = np.zeros((N, D), np.float64)
    for r in res.results:
        acc += r["out"].astype(np.float64)
    outp = (acc + bo.astype(np.float64)).astype(np.float32)
    return outp.reshape(B, T, D)



# revision 22
# speedup vs baseline: 1.1882x; 1.1882x over previous
"""Self-contained TRN2 Bass kernel for 16-head MHA (B=2, T=2048, D=1024),
head-parallel across 8 NeuronCores (2 heads per core).

kernel(**inputs) takes the FULL fp32 inputs of reference.setup_inputs() and
returns the FULL [2, 2048, 1024] fp32 output.  Host-side prep: q/k/v are
transposed to [1024, 4096] bf16 (shared by all cores); each core gets its
128-column slice of Wq/Wk/Wv (and 128-row slice of Wo) in bf16.  Each core
computes its two heads end-to-end (QKV projections, softmax attention with
row-group-packed score matmuls, ones-augmented PV for free softmax sums,
output projection) and DMAs a rank-128 partial of the output back; the host
sums the 8 partials and adds the output bias.
"""

import numpy as np

import concourse.bass as bass
import concourse.mybir as mybir
import concourse.tile as tile
from concourse import bacc

FP32 = mybir.dt.float32
BF16 = mybir.dt.bfloat16

D = 1024          # model dim
N = 4096          # B*T tokens
B = 2
T = 2048
PH = 128          # per-core projection dims (2 heads x 64)
DH = 64           # head dim
KC = 8            # contraction chunks (1024 / 128)
NTC = N // 128    # 32 token chunks of 128
SCALE = 0.125     # 1/sqrt(64)

ACT_EXP = mybir.ActivationFunctionType.Exp


def build(nc=None):
    if nc is None:
        nc = bacc.Bacc(
            "TRN2",
            target_bir_lowering=False,
            debug=False,
            enable_asserts=False,
            num_devices=8,
        )

    qT = nc.dram_tensor("qT", [D, N], BF16, kind="ExternalInput")
    kT = nc.dram_tensor("kT", [D, N], BF16, kind="ExternalInput")
    vT = nc.dram_tensor("vT", [D, N], BF16, kind="ExternalInput")
    # host pre-arranges W* into the [128, KC, 128] SBUF layout so the
    # weight DMA is one contiguous 2 KB-per-partition copy
    wq = nc.dram_tensor("wq", [128, KC * PH], BF16, kind="ExternalInput")
    wk = nc.dram_tensor("wk", [128, KC * PH], BF16, kind="ExternalInput")
    wv = nc.dram_tensor("wv", [128, KC * PH], BF16, kind="ExternalInput")
    wo = nc.dram_tensor("wo", [PH, D], BF16, kind="ExternalInput")
    bq = nc.dram_tensor("bq", [PH, 1], FP32, kind="ExternalInput")
    bk = nc.dram_tensor("bk", [PH, 1], FP32, kind="ExternalInput")
    bv = nc.dram_tensor("bv", [PH, 1], FP32, kind="ExternalInput")
    out = nc.dram_tensor("out", [N, D], FP32, kind="ExternalOutput")

    with tile.TileContext(nc) as tc:
        _emit(nc, tc, qT, kT, vT, wq, wk, wv, wo, bq, bk, bv, out)

    nc.compile()
    return nc


class _Ctx:
    pass


def _emit(nc, tc, qT, kT, vT, wq, wk, wv, wo, bq, bk, bv, out):
    from contextlib import ExitStack

    E = _Ctx()
    E.nc = nc
    E.pending = []

    ctxmgr = ExitStack()
    with ctxmgr:
        const_pool = ctxmgr.enter_context(tc.tile_pool(name="const", bufs=1))
        E.xt_pool = ctxmgr.enter_context(tc.tile_pool(name="xt", bufs=18))
        big_pool = ctxmgr.enter_context(tc.tile_pool(name="big", bufs=1))
        E.pt_pool = ctxmgr.enter_context(tc.tile_pool(name="pt", bufs=17))
        E.bc_pool = ctxmgr.enter_context(tc.tile_pool(name="bc", bufs=4))
        E.ostg_pool = ctxmgr.enter_context(tc.tile_pool(name="ostg", bufs=4))
        # PSUM: shared proj/outproj pool 2 banks + st 4 + ctx 2 = 8
        E.po_ps = ctxmgr.enter_context(
            tc.tile_pool(name="po_ps", bufs=2, space="PSUM"))
        E.st_ps = ctxmgr.enter_context(
            tc.tile_pool(name="st_ps", bufs=2, space="PSUM"))
        E.ctx_ps = ctxmgr.enter_context(
            tc.tile_pool(name="ctx_ps", bufs=2, space="PSUM"))

        # --- weights / consts to SBUF ---
        wq_sb = const_pool.tile([128, KC, PH], BF16, tag="wq")
        wk_sb = const_pool.tile([128, KC, PH], BF16, tag="wk")
        wv_sb = const_pool.tile([128, KC, PH], BF16, tag="wv")
        E.wo_sb = const_pool.tile([128, D], BF16, tag="wo")
        bq_sb = const_pool.tile([128, 1], FP32, tag="bq")
        bk_sb = const_pool.tile([128, 1], FP32, tag="bk")
        E.bv_sb = const_pool.tile([128, 1], FP32, tag="bv")
        nc.sync.dma_start(wk_sb[:], wk.ap().rearrange("p (c m) -> p c m", c=KC))
        nc.sync.dma_start(wq_sb[:], wq.ap().rearrange("p (c m) -> p c m", c=KC))
        nc.sync.dma_start(wv_sb[:], wv.ap().rearrange("p (c m) -> p c m", c=KC))
        nc.sync.dma_start(E.wo_sb[:], wo.ap())
        nc.sync.dma_start(bq_sb[:], bq.ap())
        nc.sync.dma_start(bk_sb[:], bk.ap())
        nc.sync.dma_start(E.bv_sb[:], bv.ap())

        # persistent activations
        E.qT_sb = big_pool.tile([128, N], BF16, tag="qTsb")
        E.kT_sb = big_pool.tile([128, N], BF16, tag="kTsb")
        # v_aug pair layout: [tok part, 32 tok chunks, 130]; per head h the
        # PV stationary operand is vp[:, chunk, 65h : 65h+65] = [v_h | ones]
        E.vp = big_pool.tile([128, NTC, 130], BF16, tag="vp")
        E.vT_sb = big_pool.tile([128, N], BF16, tag="vTsb")
        E.ctxT = big_pool.tile([128, N], BF16, tag="ctxT")

        nc.gpsimd.memset(E.vp[:, :, 64], 1.0)
        nc.gpsimd.memset(E.vp[:, :, 129], 1.0)

        E.identity = const_pool.tile([128, 128], BF16, tag="ident")
        from concourse.masks import make_identity
        make_identity(nc, E.identity[:])

        def dma_in(nm, xdram, b):
            lst = []
            for kc in range(KC):
                xt = E.xt_pool.tile(
                    [128, T], BF16, tag="xt", name=f"xt_{nm}{b}{kc}")
                nc.sync.dma_start(
                    xt[:], xdram.ap()[kc * 128:(kc + 1) * 128, b * T:(b + 1) * T])
                lst.append(xt)
            return lst

        def proj4(xts, wsb, bias_sb, dstT, b, ts=range(4), drain_act=False):
            for t in ts:
                _proj_chunk(E, xts, wsb, bias_sb, dstT, b * T, t, drain_act)

        def vproj(xts, b):
            proj4(xts, wv_sb, None, E.vT_sb, b)
            for tloc in range(16):
                tcid = b * 16 + tloc
                tr = E.po_ps.tile(
                    [128, 128], BF16, tag="po", name=f"tr{tcid}")
                nc.tensor.transpose(
                    tr[:], E.vT_sb[:, tcid * 128:(tcid + 1) * 128], E.identity[:])
                nc.vector.tensor_copy(E.vp[:, tcid, 0:64], tr[:, 0:64])
                nc.vector.tensor_copy(E.vp[:, tcid, 65:129], tr[:, 64:128])

        def group(b, tqc, mid_cb=None, defer=2):
            pend = _attention_group(E, b, tqc, mid_cb)
            while len(E.pending) >= defer:
                _norm_outproj(E, *E.pending.pop(0), out)
            E.pending.append(pend)

        # batch-0 inputs + K/Q projections up front; V is emitted after the
        # first attention group so the PE stream does not stall on vT DMA.
        # batch-1 inputs/projections are staggered into batch-0's ACT-bound
        # attention groups so their DMA + PE work hide in the slack.
        xk0 = dma_in("k", kT, 0)
        xq0 = dma_in("q", qT, 0)
        xv0 = dma_in("v", vT, 0)
        proj4(xk0, wk_sb, bk_sb, E.kT_sb, 0, drain_act=True)
        proj4(xq0, wq_sb, bq_sb, E.qT_sb, 0, drain_act=True)
        xk1 = dma_in("k", kT, 1)
        group(0, 0, mid_cb=lambda: vproj(xv0, 0))
        xq1 = dma_in("q", qT, 1)
        group(0, 1)
        proj4(xk1, wk_sb, bk_sb, E.kT_sb, 1, ts=(0, 1))
        xv1 = dma_in("v", vT, 1)
        group(0, 2)
        proj4(xk1, wk_sb, bk_sb, E.kT_sb, 1, ts=(2, 3))
        proj4(xq1, wq_sb, bq_sb, E.qT_sb, 1, ts=(0, 1))
        group(0, 3)
        proj4(xq1, wq_sb, bq_sb, E.qT_sb, 1, ts=(2, 3))
        group(1, 0, mid_cb=lambda: vproj(xv1, 1))
        group(1, 1)
        group(1, 2, defer=1)
        group(1, 3, defer=1)
        while E.pending:
            _norm_outproj(E, *E.pending.pop(0), out)


def _proj_chunk(E, xts, wsb, bias_sb, dstT, btok, t, drain_act=False):
    """One 512-token projection chunk: accumulate 8 kc matmuls, drain."""
    nc = E.nc
    ps = E.po_ps.tile([128, 512], FP32, tag="po", name="ps")
    for kc in range(KC):
        nc.tensor.matmul(
            ps[:],
            wsb[:, kc, :],
            xts[kc][:, t * 512:(t + 1) * 512],
            start=(kc == 0),
            stop=(kc == KC - 1),
        )
    dst = dstT[:, btok + t * 512: btok + (t + 1) * 512]
    if drain_act:
        # ScalarE drain (idle during the head phase); Identity has a free
        # per-partition bias add
        if bias_sb is not None:
            nc.scalar.activation(
                dst, ps[:], mybir.ActivationFunctionType.Identity, bias=bias_sb[:])
        else:
            nc.scalar.activation(dst, ps[:], mybir.ActivationFunctionType.Identity)
    elif bias_sb is not None:
        nc.vector.tensor_scalar_add(dst, ps[:], bias_sb[:])
    else:
        nc.vector.tensor_copy(dst, ps[:])


def _attention_group(E, b, tqc, mid_cb=None):
    """S^T/exp/PV + sums & ctx drains for one 512-token group (both heads).

    The two heads' S^T matmuls are row-group packed: head h's K=64
    contraction occupies array rows 64h..64h+63, so the pair runs
    concurrently on the PE.  st is one flat [128, 1024] PSUM tile (2 banks,
    head h in columns 512h..512h+512) so the exp is a single contiguous
    2-D ACTIVATE over 1024 elements per partition.

    With mid_cb set, all 16 ST/exp pairs are emitted first, then mid_cb()
    (used for the V projection: ScalarE stays busy on the exps while the
    PE waits for vT's DMA), then the PV accumulation.
    """
    nc = E.nc
    btok = b * T
    tq0 = btok + tqc * 512

    sums_h = [
        E.bc_pool.tile([1, 512], FP32, tag=f"sums{h}", name=f"sums{h}")
        for h in range(2)
    ]
    ctx2 = [
        E.ctx_ps.tile([65, 512], FP32, tag="ctx", name=f"ctx{h}")
        for h in range(2)
    ]

    def st_exp(tk):
        st = E.st_ps.tile([128, 1024], FP32, tag="st", name="st")
        for h in range(2):
            nc.tensor.matmul(
                st[:, h * 512:(h + 1) * 512],
                E.kT_sb[h * 64:(h + 1) * 64,
                        btok + tk * 128: btok + (tk + 1) * 128],
                E.qT_sb[h * 64:(h + 1) * 64, tq0:tq0 + 512],
                start=True,
                stop=True,
            )
        pt = E.pt_pool.tile([128, 1024], BF16, tag="pt", name="pt")
        nc.scalar.activation(pt[:], st[:], ACT_EXP, scale=SCALE)
        return pt

    def pv(tk, pt):
        for h in range(2):
            nc.tensor.matmul(
                ctx2[h][:],
                E.vp[:, b * 16 + tk, h * 65:(h + 1) * 65],
                pt[:, h * 512:(h + 1) * 512],
                start=(tk == 0),
                stop=(tk == 15),
            )

    if mid_cb is None:
        for tk in range(16):
            pv(tk, st_exp(tk))
    else:
        pts = [st_exp(tk) for tk in range(16)]
        mid_cb()
        for tk in range(16):
            pv(tk, pts[tk])

    for h in range(2):
        # softmax sums (PSUM row 64) -> sums tile partition 0
        nc.vector.tensor_copy(sums_h[h][0:1, :], ctx2[h][64:65, :])
        # ctx drain with bf16 cast (h1 shifts base 0 -> 64)
        nc.vector.tensor_copy(
            E.ctxT[h * 64:(h + 1) * 64, tq0:tq0 + 512], ctx2[h][0:64, :])
    return (tq0, sums_h)


def _norm_outproj(E, tq0, sums_h, out):
    """Normalization + V-bias + output projection for one 512-token group."""
    nc = E.nc
    bcast = E.bc_pool.tile([128, 512], FP32, tag="bcast")
    bcb = E.bc_pool.tile([128, 512], FP32, tag="bcb")
    nc.gpsimd.partition_broadcast(bcast[0:64, :], sums_h[0][0:1, :])
    nc.gpsimd.partition_broadcast(bcb[0:64, :], sums_h[1][0:1, :])
    nc.vector.tensor_copy(bcast[64:128, :], bcb[0:64, :])
    recipb = E.bc_pool.tile([128, 512], FP32, tag="recipb")
    nc.vector.reciprocal_approx_fast(recipb[:], bcast[:])
    nc.vector.tensor_mul(E.ctxT[:, tq0:tq0 + 512], E.ctxT[:, tq0:tq0 + 512], recipb[:])
    nc.vector.tensor_scalar_add(
        E.ctxT[:, tq0:tq0 + 512], E.ctxT[:, tq0:tq0 + 512], E.bv_sb[:])

    # output projection for these 512 tokens
    for tc4 in range(4):
        t0 = tq0 + tc4 * 128
        for half in range(2):
            ops = E.po_ps.tile([128, 512], FP32, tag="po", name="ops")
            nc.tensor.matmul(
                ops[:],
                E.ctxT[:, t0:t0 + 128],
                E.wo_sb[:, half * 512:(half + 1) * 512],
                start=True,
                stop=True,
            )
            ostg = E.ostg_pool.tile([128, 512], FP32, tag="ostg")
            nc.vector.tensor_copy(ostg[:], ops[:])
            nc.sync.dma_start(
                out.ap()[t0:t0 + 128, half * 512:(half + 1) * 512], ostg[:])


# ---------------- host-side helpers ----------------

def core_inputs(q, k, v, Wq, bq_, Wk, bk_, Wv, bv_, Wo, core):
    """Build the per-core input map (numpy, host-side shard/layout prep)."""
    import ml_dtypes

    bf16 = ml_dtypes.bfloat16
    dsl = slice(core * PH, (core + 1) * PH)

    def warr(W):
        # [1024, 128] slice -> [128 part, KC*128] (kc-major per partition)
        w = W[:, dsl].reshape(KC, 128, PH).transpose(1, 0, 2).reshape(128, KC * PH)
        return np.ascontiguousarray(w).astype(bf16)

    return {
        "wq": warr(Wq),
        "wk": warr(Wk),
        "wv": warr(Wv),
        "wo": np.ascontiguousarray(Wo[dsl, :]).astype(bf16),
        "bq": np.ascontiguousarray(bq_[dsl]).reshape(PH, 1).astype(np.float32),
        "bk": np.ascontiguousarray(bk_[dsl]).reshape(PH, 1).astype(np.float32),
        "bv": np.ascontiguousarray(bv_[dsl]).reshape(PH, 1).astype(np.float32),
    }


def shared_inputs(q, k, v):
    import ml_dtypes

    bf16 = ml_dtypes.bfloat16
    qT_np = np.ascontiguousarray(q.reshape(N, D).T).astype(bf16)
    kT_np = np.ascontiguousarray(k.reshape(N, D).T).astype(bf16)
    vT_np = np.ascontiguousarray(v.reshape(N, D).T).astype(bf16)
    return {"qT": qT_np, "kT": kT_np, "vT": vT_np}


# ---------------- public entry point ----------------

_NC_CACHE = []


def _get_nc():
    if not _NC_CACHE:
        _NC_CACHE.append(build())
    return _NC_CACHE[0]


def kernel(q, k, v, Wq, bq, Wk, bk, Wv, bv, Wo, bo):
    from concourse import bass_utils

    q = np.asarray(q, np.float32)
    k = np.asarray(k, np.float32)
    v = np.asarray(v, np.float32)
    Wq, bq = np.asarray(Wq, np.float32), np.asarray(bq, np.float32)
    Wk, bk = np.asarray(Wk, np.float32), np.asarray(bk, np.float32)
    Wv, bv = np.asarray(Wv, np.float32), np.asarray(bv, np.float32)
    Wo, bo = np.asarray(Wo, np.float32), np.asarray(bo, np.float32)

    nc = _get_nc()
    shared = shared_inputs(q, k, v)
    in_maps = []
    for core in range(8):
        m = dict(shared)
        m.update(core_inputs(q, k, v, Wq, bq, Wk, bk, Wv, bv, Wo, core))
        in_maps.append(m)

    res = bass_utils.run_bass_kernel_spmd(nc, in_maps, core_ids=list(range(8)))

    acc = np.zeros((N, D), np.float64)
    for r in res.results:
        acc += r["out"].astype(np.float64)
    outp = (acc + bo.astype(np.float64)).astype(np.float32)
    return outp.reshape(B, T, D)
